# revision 7
# baseline (speedup 1.0000x reference)
"""MoE feed-forward (B=4, T=2048, DIM=1024, FF=4096, E=8, top-2) on 8 trn2 cores.

Expert-parallel: core c owns expert c (gets W1[c], W2[c]). The router is
replicated: every core computes logits/top-2 softmax for all 8192 tokens,
extracts its own expert's combine weight column, runs its expert FFN over all
tokens (masked-dense), scales by the combine weight, and an AllReduce sums the
8 per-expert contributions.

Layout: tokens live on the FREE axis, channels on partitions ("transposed"
activations), so both GEMMs consume direct slices of W1/W2 and x only needs a
host-side transpose. Output is produced transposed [DIM, N] and un-transposed
on the host.
"""

import numpy as np

B, T, DIM, FF, E = 4, 2048, 1024, 4096, 8
N = B * T                      # 8192 tokens
P = 128                        # partitions
KC = DIM // P                  # 8 contraction chunks
TC = 16                        # token chunks
TW = N // TC                   # 512 tokens per chunk
G = TW // P                    # 4 router groups (128 tokens) per chunk
FFC = FF // P                  # 32 ff chunks
DC = DIM // P                  # 8 output-dim chunks

_cache = {}


def _legalize_waits(nc):
    """Move Tile-attached semaphore waits onto standalone EventSemaphore
    instructions — this walrus build rejects instructions carrying multiple
    sync waits (e.g. LDWEIGHTS, Drain)."""
    import concourse.mybir as mybir

    moved = 0
    for bb in nc.main_func.blocks:
        insts = bb.instructions
        out = []
        for ins in insts:
            si = ins.sync_info
            waits = list(si.on_wait) if si is not None else []
            if waits:
                for k, w in enumerate(waits):
                    car = mybir.InstEventSemaphore(
                        name=f"{ins.name}_wt{k}", ins=[], outs=[]
                    )
                    car.engine = ins.engine
                    csi = car.sync_info
                    if csi is None:
                        csi = mybir.SyncInfo(on_wait=[], on_update=[])
                    csi.on_wait = [w]
                    car.sync_info = csi
                    out.append(car)
                    moved += 1
                si.on_wait = []
                ins.sync_info = si
            out.append(ins)
        while len(insts):
            insts.pop()
        for x in out:
            insts.append(x)
    return moved


def _build():
    import concourse.bass as bass
    import concourse.mybir as mybir
    import concourse.tile as tile

    fp32 = mybir.dt.float32
    AX = mybir.AxisListType
    ALU = mybir.AluOpType
    ACT = mybir.ActivationFunctionType

    nc = bass.Bass()
    xT = nc.declare_dram_parameter("xT", [DIM, N], fp32, isOutput=False)
    wrt = nc.declare_dram_parameter("wrt", [DIM, E], fp32, isOutput=False)
    w1 = nc.declare_dram_parameter("w1", [DIM, FF], fp32, isOutput=False)
    w2 = nc.declare_dram_parameter("w2", [FF, DIM], fp32, isOutput=False)
    esel = nc.declare_dram_parameter("esel", [P, E], fp32, isOutput=False)
    eye = nc.declare_dram_parameter("eye", [P, P], fp32, isOutput=False)
    out_ext = nc.declare_dram_parameter("out", [DIM, N], fp32, isOutput=True)

    with tile.TileContext(nc) as tc:
        with (
            tc.tile_pool(name="const", bufs=1) as constp,
            tc.tile_pool(name="xt", bufs=2) as xtp,
            tc.tile_pool(name="w1p", bufs=3) as w1p,
            tc.tile_pool(name="w2p", bufs=2) as w2p,
            tc.tile_pool(name="ht", bufs=FFC) as htp,
            tc.tile_pool(name="rt", bufs=4) as rtp,
            tc.tile_pool(name="yb", bufs=4) as ybp,
            tc.tile_pool(name="ps_l", bufs=2, space="PSUM") as ps_l,
            tc.tile_pool(name="ps_h", bufs=2, space="PSUM") as ps_h,
            tc.tile_pool(name="ps_y", bufs=2, space="PSUM") as ps_y,
            tc.tile_pool(name="ps_t", bufs=2, space="PSUM") as ps_t,
            tc.tile_pool(name="dram", bufs=1, space="DRAM") as dram,
            tc.tile_pool(name="dramw", bufs=2, space="DRAM") as dramw,
        ):
            # constants
            wrt_sb = constp.tile([P, KC, E], fp32)
            nc.sync.dma_start(wrt_sb[:], wrt.rearrange("(kc p) e -> p kc e", p=P))
            esel_sb = constp.tile([P, E], fp32)
            nc.sync.dma_start(esel_sb[:], esel[:, :])
            eye_sb = constp.tile([P, P], fp32)
            nc.sync.dma_start(eye_sb[:], eye[:, :])

            outb = dram.tile([DIM, N], fp32)
            outr = dram.tile([DIM, N], fp32)

            for t in range(TC):
                ts = t * TW
                # x block for this token chunk: [128, kc, 512]
                xt_sb = xtp.tile([P, KC, TW], fp32)
                nc.sync.dma_start(
                    xt_sb[:], xT[:, ts:ts + TW].rearrange("(kc p) n -> p kc n", p=P)
                )

                # ---- router for these 512 tokens -> w_et [128, G] ----
                w_et = rtp.tile([P, G], fp32)
                for g in range(G):
                    psl = ps_l.tile([P, E], fp32)
                    for kc in range(KC):
                        nc.tensor.matmul(
                            psl[:],
                            xt_sb[:, kc, g * P:(g + 1) * P],
                            wrt_sb[:, kc, :],
                            start=(kc == 0),
                            stop=(kc == KC - 1),
                        )
                    m1 = rtp.tile([P, 1], fp32)
                    nc.vector.reduce_max(m1[:], psl[:], axis=AX.X)
                    nm1 = rtp.tile([P, 1], fp32)
                    nc.scalar.mul(nm1[:], m1[:], -1.0)
                    lg = rtp.tile([P, E], fp32)
                    nc.vector.tensor_scalar(lg[:], psl[:], nm1[:], None, ALU.add)
                    msk = rtp.tile([P, E], fp32)
                    nc.vector.tensor_scalar(msk[:], lg[:], 0.0, None, ALU.is_ge)
                    lmk = rtp.tile([P, E], fp32)
                    nc.vector.tensor_scalar(lmk[:], msk[:], -1e30, None, ALU.mult)
                    nc.vector.tensor_tensor(lmk[:], lmk[:], lg[:], ALU.add)
                    m2 = rtp.tile([P, 1], fp32)
                    nc.vector.reduce_max(m2[:], lmk[:], axis=AX.X)
                    el = rtp.tile([P, E], fp32)
                    nc.scalar.activation(el[:], lg[:], ACT.Exp)
                    em2 = rtp.tile([P, 1], fp32)
                    nc.scalar.activation(em2[:], m2[:], ACT.Exp)
                    den = rtp.tile([P, 1], fp32)
                    nc.scalar.add(den[:], em2[:], 1.0)
                    rden = rtp.tile([P, 1], fp32)
                    nc.vector.reciprocal(rden[:], den[:])
                    sel = rtp.tile([P, E], fp32)
                    nc.vector.tensor_scalar(sel[:], lg[:], m2[:], None, ALU.is_ge)
                    w8 = rtp.tile([P, E], fp32)
                    nc.vector.tensor_tensor(w8[:], el[:], sel[:], ALU.mult)
                    nc.vector.tensor_scalar(w8[:], w8[:], rden[:], None, ALU.mult)
                    nc.vector.tensor_tensor(w8[:], w8[:], esel_sb[:], ALU.mult)
                    nc.vector.reduce_sum(w_et[:, g:g + 1], w8[:], axis=AX.X)

                # transpose w_et -> [G, 128] -> row [1, 512] -> bcast [128, 512]
                pswt = ps_t.tile([G, P], fp32)
                nc.tensor.transpose(pswt[:], w_et[:], eye_sb[:])
                wrow = rtp.tile([G, P], fp32)
                nc.scalar.copy(wrow[:], pswt[:])
                wdram = dramw.tile([1, TW], fp32)
                nc.sync.dma_start(wdram[0:1, :], wrow[:, :])
                wb = rtp.tile([P, TW], fp32)
                nc.sync.dma_start(wb[:], wdram[0:1, :].broadcast_to((P, TW)))

                # ---- FFN: hT = gelu(W1.T @ x) ----
                hts = []
                for ffc in range(FFC):
                    w1_sb = w1p.tile([P, KC, P], fp32)
                    nc.sync.dma_start(
                        w1_sb[:],
                        w1[:, ffc * P:(ffc + 1) * P].rearrange(
                            "(kc p) f -> p kc f", p=P
                        ),
                    )
                    ph = ps_h.tile([P, TW], fp32)
                    for kc in range(KC):
                        nc.tensor.matmul(
                            ph[:],
                            w1_sb[:, kc, :],
                            xt_sb[:, kc, :],
                            start=(kc == 0),
                            stop=(kc == KC - 1),
                        )
                    ht = htp.tile([P, TW], fp32, tag="ht")
                    nc.scalar.activation(ht[:], ph[:], ACT.Gelu)
                    hts.append(ht)

                # ---- yT = W2.T @ hT, scaled by combine weight ----
                for dc in range(DC):
                    w2_sb = w2p.tile([P, FFC, P], fp32)
                    nc.sync.dma_start(
                        w2_sb[:],
                        w2[:, dc * P:(dc + 1) * P].rearrange(
                            "(fc p) d -> p fc d", p=P
                        ),
                    )
                    py = ps_y.tile([P, TW], fp32)
                    for fc in range(FFC):
                        nc.tensor.matmul(
                            py[:],
                            w2_sb[:, fc, :],
                            hts[fc][:],
                            start=(fc == 0),
                            stop=(fc == FFC - 1),
                        )
                    ysb = ybp.tile([P, TW], fp32)
                    nc.vector.tensor_tensor(ysb[:], py[:], wb[:], ALU.mult)
                    nc.sync.dma_start(
                        outb[dc * P:(dc + 1) * P, ts:ts + TW], ysb[:]
                    )

            nc.gpsimd.collective_compute(
                "AllReduce",
                mybir.AluOpType.add,
                ins=[outb.opt()],
                outs=[outr.opt()],
                replica_groups=[list(range(8))],
            )
            nc.sync.dma_start(out_ext[:, :], outr[:, :])

    _legalize_waits(nc)
    return nc


def kernel(x, Wr, W1, W2):
    from concourse.bass_utils import run_bass_kernel_spmd

    if "nc" not in _cache:
        _cache["nc"] = _build()
    nc = _cache["nc"]

    xTf = np.ascontiguousarray(x.reshape(N, DIM).T.astype(np.float32))
    wrt = np.ascontiguousarray(Wr.T.astype(np.float32))
    eye = np.eye(P, dtype=np.float32)
    in_maps = []
    for c in range(8):
        esel = np.zeros((P, E), dtype=np.float32)
        esel[:, c] = 1.0
        in_maps.append({
            "xT": xTf,
            "wrt": wrt,
            "w1": np.ascontiguousarray(W1[c].astype(np.float32)),
            "w2": np.ascontiguousarray(W2[c].astype(np.float32)),
            "esel": esel,
            "eye": eye,
        })
    res = run_bass_kernel_spmd(nc, in_maps, list(range(8)))
    _cache["last_result"] = res
    out = res.results[0]["out"]          # [DIM, N]
    return np.ascontiguousarray(out.T).reshape(B, T, DIM).astype(np.float32)


# revision 11
# speedup vs baseline: 1.1126x; 1.1126x over previous
"""MoE feed-forward (B=4, T=2048, DIM=1024, FF=4096, E=8, top-2) on 8 trn2 cores.

Expert-parallel: core c owns expert c (gets W1[c], W2[c]). The router is
replicated: every core computes logits/top-2 softmax for all 8192 tokens,
extracts its own expert's combine weight column, runs its expert FFN over all
tokens (masked-dense), scales by the combine weight, and an AllReduce sums the
8 per-expert contributions.

Layout: tokens live on the FREE axis, channels on partitions ("transposed"
activations), so both GEMMs consume direct slices of W1/W2 and x only needs a
host-side transpose. Output is produced transposed [DIM, N] and un-transposed
on the host.
"""

import numpy as np

B, T, DIM, FF, E = 4, 2048, 1024, 4096, 8
N = B * T                      # 8192 tokens
P = 128                        # partitions
KC = DIM // P                  # 8 contraction chunks
TC = 16                        # token chunks
TW = N // TC                   # 512 tokens per chunk
G = TW // P                    # 4 router groups (128 tokens) per chunk
FFC = FF // P                  # 32 ff chunks
DC = DIM // P                  # 8 output-dim chunks

_cache = {}


def _legalize_waits(nc):
    """Move Tile-attached semaphore waits onto standalone EventSemaphore
    instructions — this walrus build rejects instructions carrying multiple
    sync waits (e.g. LDWEIGHTS, Drain)."""
    import concourse.mybir as mybir

    moved = 0
    for bb in nc.main_func.blocks:
        insts = bb.instructions
        out = []
        for ins in insts:
            si = ins.sync_info
            waits = list(si.on_wait) if si is not None else []
            if waits:
                for k, w in enumerate(waits):
                    car = mybir.InstEventSemaphore(
                        name=f"{ins.name}_wt{k}", ins=[], outs=[]
                    )
                    car.engine = ins.engine
                    csi = car.sync_info
                    if csi is None:
                        csi = mybir.SyncInfo(on_wait=[], on_update=[])
                    csi.on_wait = [w]
                    car.sync_info = csi
                    out.append(car)
                    moved += 1
                si.on_wait = []
                ins.sync_info = si
            out.append(ins)
        while len(insts):
            insts.pop()
        for x in out:
            insts.append(x)
    return moved


def _build():
    import concourse.bass as bass
    import concourse.mybir as mybir
    import concourse.tile as tile

    fp32 = mybir.dt.float32
    AX = mybir.AxisListType
    ALU = mybir.AluOpType
    ACT = mybir.ActivationFunctionType

    nc = bass.Bass()
    xT = nc.declare_dram_parameter("xT", [DIM, N], fp32, isOutput=False)
    wrt = nc.declare_dram_parameter("wrt", [DIM, E], fp32, isOutput=False)
    w1 = nc.declare_dram_parameter("w1", [DIM, FF], fp32, isOutput=False)
    w2 = nc.declare_dram_parameter("w2", [FF, DIM], fp32, isOutput=False)
    esel = nc.declare_dram_parameter("esel", [P, E], fp32, isOutput=False)
    eye = nc.declare_dram_parameter("eye", [P, P], fp32, isOutput=False)
    out_ext = nc.declare_dram_parameter("out", [DIM // 8, N], fp32, isOutput=True)

    with tile.TileContext(nc) as tc:
        with (
            tc.tile_pool(name="const", bufs=1) as constp,
            tc.tile_pool(name="xt", bufs=2) as xtp,
            tc.tile_pool(name="w1p", bufs=3) as w1p,
            tc.tile_pool(name="w2p", bufs=2) as w2p,
            tc.tile_pool(name="ht", bufs=FFC) as htp,
            tc.tile_pool(name="rt", bufs=4) as rtp,
            tc.tile_pool(name="yb", bufs=4) as ybp,
            tc.tile_pool(name="ps_l", bufs=2, space="PSUM") as ps_l,
            tc.tile_pool(name="ps_h", bufs=2, space="PSUM") as ps_h,
            tc.tile_pool(name="ps_y", bufs=2, space="PSUM") as ps_y,
            tc.tile_pool(name="ps_t", bufs=2, space="PSUM") as ps_t,
            tc.tile_pool(name="dram", bufs=1, space="DRAM") as dram,
            tc.tile_pool(name="dramw", bufs=2, space="DRAM") as dramw,
        ):
            # constants
            wrt_sb = constp.tile([P, KC, E], fp32)
            nc.sync.dma_start(wrt_sb[:], wrt.rearrange("(kc p) e -> p kc e", p=P))
            esel_sb = constp.tile([P, E], fp32)
            nc.sync.dma_start(esel_sb[:], esel[:, :])
            eye_sb = constp.tile([P, P], fp32)
            nc.sync.dma_start(eye_sb[:], eye[:, :])

            outb = dram.tile([DIM, N], fp32)
            outr = dram.tile([DIM // 8, N], fp32)

            for t in range(TC):
                ts = t * TW
                # x block for this token chunk: [128, kc, 512]
                xt_sb = xtp.tile([P, KC, TW], fp32)
                nc.sync.dma_start(
                    xt_sb[:], xT[:, ts:ts + TW].rearrange("(kc p) n -> p kc n", p=P)
                )

                # ---- router for these 512 tokens -> w_et [128, G] ----
                w_et = rtp.tile([P, G], fp32)
                for g in range(G):
                    psl = ps_l.tile([P, E], fp32)
                    for kc in range(KC):
                        nc.tensor.matmul(
                            psl[:],
                            xt_sb[:, kc, g * P:(g + 1) * P],
                            wrt_sb[:, kc, :],
                            start=(kc == 0),
                            stop=(kc == KC - 1),
                        )
                    m1 = rtp.tile([P, 1], fp32)
                    nc.vector.reduce_max(m1[:], psl[:], axis=AX.X)
                    nm1 = rtp.tile([P, 1], fp32)
                    nc.scalar.mul(nm1[:], m1[:], -1.0)
                    lg = rtp.tile([P, E], fp32)
                    nc.vector.tensor_scalar(lg[:], psl[:], nm1[:], None, ALU.add)
                    msk = rtp.tile([P, E], fp32)
                    nc.vector.tensor_scalar(msk[:], lg[:], 0.0, None, ALU.is_ge)
                    lmk = rtp.tile([P, E], fp32)
                    nc.vector.tensor_scalar(lmk[:], msk[:], -1e30, None, ALU.mult)
                    nc.vector.tensor_tensor(lmk[:], lmk[:], lg[:], ALU.add)
                    m2 = rtp.tile([P, 1], fp32)
                    nc.vector.reduce_max(m2[:], lmk[:], axis=AX.X)
                    el = rtp.tile([P, E], fp32)
                    nc.scalar.activation(el[:], lg[:], ACT.Exp)
                    em2 = rtp.tile([P, 1], fp32)
                    nc.scalar.activation(em2[:], m2[:], ACT.Exp)
                    den = rtp.tile([P, 1], fp32)
                    nc.scalar.add(den[:], em2[:], 1.0)
                    rden = rtp.tile([P, 1], fp32)
                    nc.vector.reciprocal(rden[:], den[:])
                    sel = rtp.tile([P, E], fp32)
                    nc.vector.tensor_scalar(sel[:], lg[:], m2[:], None, ALU.is_ge)
                    w8 = rtp.tile([P, E], fp32)
                    nc.vector.tensor_tensor(w8[:], el[:], sel[:], ALU.mult)
                    nc.vector.tensor_scalar(w8[:], w8[:], rden[:], None, ALU.mult)
                    nc.vector.tensor_tensor(w8[:], w8[:], esel_sb[:], ALU.mult)
                    nc.vector.reduce_sum(w_et[:, g:g + 1], w8[:], axis=AX.X)

                # transpose w_et -> [G, 128] -> row [1, 512] -> bcast [128, 512]
                pswt = ps_t.tile([G, P], fp32)
                nc.tensor.transpose(pswt[:], w_et[:], eye_sb[:])
                wrow = rtp.tile([G, P], fp32)
                nc.scalar.copy(wrow[:], pswt[:])
                wdram = dramw.tile([1, TW], fp32)
                nc.sync.dma_start(wdram[0:1, :], wrow[:, :])
                wb = rtp.tile([P, TW], fp32)
                nc.sync.dma_start(wb[:], wdram[0:1, :].broadcast_to((P, TW)))

                # ---- FFN: hT = gelu(W1.T @ x) ----
                hts = []
                for ffc in range(FFC):
                    w1_sb = w1p.tile([P, KC, P], fp32)
                    nc.sync.dma_start(
                        w1_sb[:],
                        w1[:, ffc * P:(ffc + 1) * P].rearrange(
                            "(kc p) f -> p kc f", p=P
                        ),
                    )
                    ph = ps_h.tile([P, TW], fp32)
                    for kc in range(KC):
                        nc.tensor.matmul(
                            ph[:],
                            w1_sb[:, kc, :],
                            xt_sb[:, kc, :],
                            start=(kc == 0),
                            stop=(kc == KC - 1),
                        )
                    ht = htp.tile([P, TW], fp32, tag="ht")
                    nc.scalar.activation(ht[:], ph[:], ACT.Gelu)
                    hts.append(ht)

                # ---- yT = W2.T @ hT, scaled by combine weight ----
                for dc in range(DC):
                    w2_sb = w2p.tile([P, FFC, P], fp32)
                    nc.sync.dma_start(
                        w2_sb[:],
                        w2[:, dc * P:(dc + 1) * P].rearrange(
                            "(fc p) d -> p fc d", p=P
                        ),
                    )
                    py = ps_y.tile([P, TW], fp32)
                    for fc in range(FFC):
                        nc.tensor.matmul(
                            py[:],
                            w2_sb[:, fc, :],
                            hts[fc][:],
                            start=(fc == 0),
                            stop=(fc == FFC - 1),
                        )
                    ysb = ybp.tile([P, TW], fp32)
                    nc.vector.tensor_tensor(ysb[:], py[:], wb[:], ALU.mult)
                    nc.sync.dma_start(
                        outb[dc * P:(dc + 1) * P, ts:ts + TW], ysb[:]
                    )

            nc.gpsimd.collective_compute(
                "ReduceScatter",
                mybir.AluOpType.add,
                ins=[outb.opt()],
                outs=[outr.opt()],
                replica_groups=[list(range(8))],
            )
            nc.sync.dma_start(out_ext[:, :], outr[:, :])

    _legalize_waits(nc)
    return nc


def kernel(x, Wr, W1, W2):
    from concourse.bass_utils import run_bass_kernel_spmd

    if "nc" not in _cache:
        _cache["nc"] = _build()
    nc = _cache["nc"]

    xTf = np.ascontiguousarray(x.reshape(N, DIM).T.astype(np.float32))
    wrt = np.ascontiguousarray(Wr.T.astype(np.float32))
    eye = np.eye(P, dtype=np.float32)
    in_maps = []
    for c in range(8):
        esel = np.zeros((P, E), dtype=np.float32)
        esel[:, c] = 1.0
        in_maps.append({
            "xT": xTf,
            "wrt": wrt,
            "w1": np.ascontiguousarray(W1[c].astype(np.float32)),
            "w2": np.ascontiguousarray(W2[c].astype(np.float32)),
            "esel": esel,
            "eye": eye,
        })
    res = run_bass_kernel_spmd(nc, in_maps, list(range(8)))
    _cache["last_result"] = res
    # ReduceScatter: core c holds the summed rows [c*128:(c+1)*128] of [DIM, N]
    out = np.concatenate([res.results[c]["out"] for c in range(8)], axis=0)
    return np.ascontiguousarray(out.T).reshape(B, T, DIM).astype(np.float32)
